# revision 25
# baseline (speedup 1.0000x reference)
"""CAAN attention-scorer kernel for 8 Trainium2 NeuronCores (v2).

scores = relu(softmax(QK^T/sqrt(D)) @ V @ W1 + b1) @ W2 + b2
with Q/K/V = x @ W{q,k,v} + b{q,k,v};  N=8192, IN_DIM=1024, D=512.

Math restructure vs v1 (all exact up to fp8 rounding):
  * Wv@W1 folded on host (attention rows sum to 1):
      V'' = x @ (Wv W1) + (bv W1 + b1)  [N, 256]
      h   = relu(attn @ V'')            -- the whole first MLP layer is gone
  * bk dropped: a per-row constant shift of the logits cancels in softmax.
  * softmax normalization deferred past relu (relu(a)/c == relu(a/c), c>0)
    to the final [1, N] score vector: scores = (W2/2)^T relu(ctxU'')*recip.

Sharding: row-parallel attention; each core owns 1024 query rows. The
kT / V'' projections are REPLICATED on every core (EXCHANGE=False, the
shipped configuration): measured on this axon/PJRT runtime, the NRT
AllGather has a ~190us serial latency even for tiny payloads (it overlaps
compute but is useless as a data dependency), and SBUF-to-SBUF
remote_dma_broadcast does not deliver at all (sem-only broadcast timed
out and left the device NRT_EXEC_UNIT_UNRECOVERABLE) — so replicating
the 1/8-sized folded projections is the fastest working option. The
EXCHANGE=True path (XOR-addressed remote_dma_broadcast all-gather with a
ready-handshake) is kept for a runtime where SWDGE remote DMA works.

Tile's scheduling simulator cannot model remotely-incremented semaphores,
so waits on them are emitted with threshold 0 and patched to the real
value after TileContext exits (before finalize).

Layouts (x8 / x16 are exact power-of-two fp8 pre-scales):
  kT[d, j]   = (8 Wk)^T x^T             d on partitions   (fp8)
  qT[d, i]   = (8 Wq)^T xq^T (+8 bq)                      (fp8)
  v''[j, d'] = x (16 WvW1) (+16 b1'')   j on partitions   (fp8)
  S^T[j, i]  = kT-chunk^T qT            (= 64 * true)
  E          = exp(S^T / (64 sqrt(D)))  PSUM -> SBUF fp8, [P, 1024] ops
  ctxU''    += v''-chunk^T E            (= 16 * true)
  den[1,i]  += (8.0)^T E                (= 8 * sum E)
  raw[1, i]  = (W2/2)^T relu(ctxU'')    (= 8 * true raw)
  out        = raw * (1/den) + b2
"""

import numpy as np
import ml_dtypes

import concourse.tile as tile
from concourse import bacc, mybir
from concourse.bass_utils import run_bass_kernel_spmd

N, IN_DIM, D = 8192, 1024, 512
DP2 = D // 2                # 256 folded v'' width
NCORES = 8
NB = N // NCORES            # 1024 rows per core
P = 128
KC = IN_DIM // P            # 8 k-chunks of the input dim
KP = KC // 2                # 4 DoubleRow k-pairs
DC = D // P                 # 4 d-chunks (q/k)
DPAIR = DC // 2             # 2 DoubleRow d-pairs
VC = DP2 // P               # 2 d'-chunks (v'')
JT = 512                    # j-tile width in phase A
BJC = NB // P               # 8 j-chunks per block
BJP = BJC // 2              # 4 j-pairs per block
IH = 512                    # i-half width in phase B
NIH = NB // IH              # 2
WS = 8.0                    # fp8 weight pre-scale for Wq/Wk
WSV = 16.0                  # fp8 pre-scale for folded WvW1
SCALE = 1.0 / float(np.sqrt(np.float32(D))) / (WS * WS)

FP8 = mybir.dt.float8e4
BF16 = mybir.dt.bfloat16
F32 = mybir.dt.float32
DR = mybir.MatmulPerfMode.DoubleRow
ACT = mybir.ActivationFunctionType

EXCHANGE = False

_CACHE = {}


def _build(exchange=EXCHANGE):
    nc = bacc.Bacc(None, target_bir_lowering=False, debug=False,
                   num_devices=NCORES, use_seq_codegen=True)

    NT = N // JT
    xq = nc.declare_dram_parameter("xq", [P, KC, NB], FP8, isOutput=False)
    if not exchange:
        # chunk-major so each 512-column tile is one contiguous DMA
        xt = nc.declare_dram_parameter("xt", [NT, P, KC, JT], FP8,
                                       isOutput=False)
    wq = nc.declare_dram_parameter("wq", [P, KC, D], FP8, isOutput=False)
    wk = nc.declare_dram_parameter("wk", [P, KC, D], FP8, isOutput=False)
    wv1 = nc.declare_dram_parameter("wv1", [P, KC, DP2], FP8, isOutput=False)
    bq = nc.declare_dram_parameter("bq", [P, DC], F32, isOutput=False)   # x8
    bvp = nc.declare_dram_parameter("bvp", [P, 4, DP2], F32,
                                    isOutput=False)  # x16
    w2 = nc.declare_dram_parameter("w2", [P, VC], BF16, isOutput=False)  # x0.5
    b2 = nc.declare_dram_parameter("b2", [1, 1], F32, isOutput=False)
    out = nc.declare_dram_parameter("out", [1, NB], F32, isOutput=True)

    patches = []   # (BassInstruction, real_wait_value)
    my_sems = []
    with tile.TileContext(nc) as tc:
        with tc.tile_pool(name="singles", bufs=1) as singles:
            # ---- weights / constants into SBUF ----
            wk_sb = singles.tile([P, KC, D], FP8)
            wq_sb = singles.tile([P, KC, D], FP8)
            wv1_sb = singles.tile([P, KC, DP2], FP8)
            bq_sb = singles.tile([P, DC], F32)
            bvp_sb = singles.tile([P, 4, DP2], F32)
            w2_sb = singles.tile([P, VC], BF16)
            b2_sb = singles.tile([1, 1], F32)
            cs_w = singles.tile([P, 2, 32], FP8)   # colsum weights = 8.0
            xq_sb = singles.tile([P, KC, NB], FP8)
            if not exchange:
                xt_sb = singles.tile([P, NT, KC, JT], FP8)
            qt_sb = singles.tile([P, DC, NB], FP8)
            # gathered kT / v'': slot s = block of core (own XOR s)
            # (global block s when exchange=False)
            ktg = singles.tile([P, NCORES, DC, NB], FP8)
            vg = singles.tile([P, NCORES, BJC, DP2], FP8)
            out_sb = singles.tile([1, NB], F32)

            # DMA order tuned so Q can start immediately and kt follows:
            # xq (sync) + wq/bq (gpsimd) land first, wk next, then xt.
            for h in range(2):
                nc.sync.dma_start(xq_sb[:, 4 * h:4 * h + 4],
                                  xq[:, 4 * h:4 * h + 4])
            for h in range(2):
                nc.gpsimd.dma_start(out=wq_sb[:, 4 * h:4 * h + 4],
                                    in_=wq[:, 4 * h:4 * h + 4])
            for h in range(2):
                nc.gpsimd.dma_start(out=wk_sb[:, 4 * h:4 * h + 4],
                                    in_=wk[:, 4 * h:4 * h + 4])
            nc.gpsimd.dma_start(out=bq_sb[:], in_=bq[:])
            nc.gpsimd.dma_start(out=wv1_sb[:], in_=wv1[:])
            nc.gpsimd.dma_start(out=bvp_sb[:], in_=bvp[:])
            nc.gpsimd.dma_start(out=w2_sb[:], in_=w2[:])
            nc.gpsimd.dma_start(out=b2_sb[:], in_=b2[:])
            nc.vector.memset(cs_w[:], WS)
            if not exchange:
                for t in range(NT):
                    nc.sync.dma_start(xt_sb[:, t], xt[t])

            if exchange:
                rsems = [nc.alloc_semaphore(f"xch_{m}")
                         for m in range(1, NCORES)]
                ready = nc.alloc_semaphore("xch_ready")
                lsem = nc.alloc_semaphore("xch_local")
                my_sems = rsems + [ready, lsem]
                margin_dram = nc.dram_tensor("margin_scratch", [1, 2], FP8)

            # j-source slices for the kT / v'' projections
            if exchange:
                kv_blocks = 1

                def k_mov(t, kp):
                    return xq_sb[:, 2 * kp:2 * kp + 2,
                                 (t % 2) * JT:(t % 2 + 1) * JT]

                def v_mov(t, kp, q):
                    jc = (t % 2) * 4 + q
                    return xq_sb[:, 2 * kp:2 * kp + 2, jc * P:(jc + 1) * P]
            else:
                kv_blocks = NCORES

                def k_mov(t, kp):
                    return xt_sb[:, t, 2 * kp:2 * kp + 2]

                def v_mov(t, kp, q):
                    return xt_sb[:, t, 2 * kp:2 * kp + 2, q * P:(q + 1) * P]

            with (
                tc.tile_pool(name="st2", bufs=2, space="PSUM") as st2_pool,
                tc.tile_pool(name="ctxp", bufs=1, space="PSUM") as ctx_pool,
                tc.tile_pool(name="csp", bufs=1, space="PSUM") as cs_pool,
                tc.tile_pool(name="scp", bufs=1, space="PSUM") as sc_pool,
                tc.tile_pool(name="etile", bufs=8) as etile,
                tc.tile_pool(name="mlp", bufs=3) as mlp,
            ):
                # ---- phase A ----
                def emit_q():
                    # qT (+8 bq): only needs xq/wq
                    for it in range(NB // JT):
                        for dcp in range(DPAIR):
                            ps = st2_pool.tile([P, 2, JT], F32, tag="st")
                            for half in range(2):
                                dc = 2 * dcp + half
                                for kp in range(KP):
                                    nc.tensor.matmul(
                                        ps[:, half],
                                        wq_sb[:, 2 * kp:2 * kp + 2,
                                              dc * P:(dc + 1) * P],
                                        xq_sb[:, 2 * kp:2 * kp + 2,
                                              it * JT:(it + 1) * JT],
                                        start=(kp == 0), stop=(kp == KP - 1),
                                        perf_mode=DR)
                            for half in range(2):
                                dc = 2 * dcp + half
                                nc.scalar.activation(
                                    qt_sb[:, dc, it * JT:(it + 1) * JT],
                                    ps[:, half], ACT.Identity,
                                    bias=bq_sb[:, dc:dc + 1], scale=1.0)

                def emit_kv():
                    # kT / v'' per 512-column tile, in xt DMA order
                    for b in range(kv_blocks):
                        for jh in range(NB // JT):
                            t = 2 * b + jh
                            # kT (no bias; bk cancels in softmax)
                            for dcp in range(DPAIR):
                                ps = st2_pool.tile([P, 2, JT], F32, tag="st")
                                for half in range(2):
                                    dc = 2 * dcp + half
                                    for kp in range(KP):
                                        nc.tensor.matmul(
                                            ps[:, half],
                                            wk_sb[:, 2 * kp:2 * kp + 2,
                                                  dc * P:(dc + 1) * P],
                                            k_mov(t, kp),
                                            start=(kp == 0),
                                            stop=(kp == KP - 1),
                                            perf_mode=DR)
                                nc.scalar.activation(
                                    ktg[:, b, 2 * dcp:2 * dcp + 2,
                                        jh * JT:(jh + 1) * JT],
                                    ps[:], ACT.Copy, bias=0.0, scale=1.0)
                            # v'' for the same 4 j-chunks
                            ps = st2_pool.tile([P, 4, DP2], F32, tag="st")
                            for q in range(4):
                                for kp in range(KP):
                                    nc.tensor.matmul(
                                        ps[:, q],
                                        v_mov(t, kp, q),
                                        wv1_sb[:, 2 * kp:2 * kp + 2],
                                        start=(kp == 0), stop=(kp == KP - 1),
                                        perf_mode=DR)
                            nc.vector.tensor_tensor(
                                vg[:, b, 4 * jh:4 * jh + 4], ps[:],
                                bvp_sb[:], mybir.AluOpType.add)

                if exchange:
                    # kv shards first so the sends fire early; Q fills the
                    # window while data is in flight
                    emit_kv()
                    # ready handshake (NO self-send) + 14 XOR sends
                    nc.gpsimd.dma_start(margin_dram[:], ktg[0:1, 0, 0, 0:2])
                    nc.gpsimd.remote_sem_update_broadcast(
                        remote_sem=ready, local_sem=lsem,
                        rdests=[None] + [(0, k) for k in range(1, NCORES)])
                    nc.gpsimd.trigger_dma(count=1)
                    for m in range(1, NCORES):
                        rdests = [None] * 8
                        rdests[m] = (0, m)
                        nc.gpsimd.remote_dma_broadcast(
                            ktg[:, m], ktg[:, 0],
                            remote_sem=rsems[m - 1], local_sem=lsem,
                            rdests=rdests)
                        nc.gpsimd.remote_dma_broadcast(
                            vg[:, m], vg[:, 0],
                            remote_sem=rsems[m - 1], local_sem=lsem,
                            rdests=rdests)
                    # 7 peers x (+2) = 14: everyone (but me) has entered
                    w = nc.gpsimd.wait_ge(ready, 0)
                    patches.append((w, 2 * (NCORES - 1)))
                    nc.gpsimd.trigger_dma(count=None)
                    emit_q()
                else:
                    emit_q()
                    emit_kv()

                # ---- phase B: attention over the 8 blocks ----
                for ih in range(NIH):
                    i0 = ih * IH
                    ctx_ps = ctx_pool.tile([P, VC, IH], F32)
                    cs_ps = cs_pool.tile([32, IH], F32)
                    for s in range(NCORES):
                        if exchange and ih == 0 and s >= 1:
                            w = nc.tensor.wait_ge(rsems[s - 1], 0)
                            patches.append((w, 4))
                        for tp in range(BJP):
                            st = st2_pool.tile([P, 2, IH], F32, tag="st")
                            for half in range(2):
                                jc = 2 * tp + half
                                for dp in range(DPAIR):
                                    nc.tensor.matmul(
                                        st[:, half],
                                        ktg[:, s, 2 * dp:2 * dp + 2,
                                            jc * P:(jc + 1) * P],
                                        qt_sb[:, 2 * dp:2 * dp + 2,
                                              i0:i0 + IH],
                                        start=(dp == 0),
                                        stop=(dp == DPAIR - 1),
                                        perf_mode=DR)
                            e_t = etile.tile([P, 2, IH], FP8, tag="et")
                            nc.scalar.activation(e_t[:], st[:], ACT.Exp,
                                                 bias=0.0, scale=SCALE)
                            first = (s == 0 and tp == 0)
                            last = (s == NCORES - 1 and tp == BJP - 1)
                            nc.tensor.matmul(cs_ps[:], cs_w[:], e_t[:],
                                             start=first, stop=last,
                                             perf_mode=DR)
                            for vc in range(VC):
                                nc.tensor.matmul(
                                    ctx_ps[:, vc],
                                    vg[:, s, 2 * tp:2 * tp + 2,
                                       vc * P:(vc + 1) * P],
                                    e_t[:],
                                    start=first, stop=last,
                                    perf_mode=DR)

                    # tail: h = relu(ctxU''); raw = (W2/2)^T h; out = raw/den
                    h_sb = mlp.tile([P, VC, IH], BF16, tag="hsb")
                    sc_ps = sc_pool.tile([1, IH], F32, tag="sc")
                    for vc in range(VC):
                        nc.scalar.activation(h_sb[:, vc], ctx_ps[:, vc],
                                             ACT.Relu, bias=0.0, scale=1.0)
                        nc.tensor.matmul(sc_ps[:], w2_sb[:, vc:vc + 1],
                                         h_sb[:, vc],
                                         start=(vc == 0), stop=(vc == VC - 1))
                    recip = mlp.tile([1, IH], F32, tag="recip")
                    nc.vector.reciprocal_approx_fast(recip[:], cs_ps[0:1])
                    rawn = mlp.tile([1, IH], F32, tag="rawn")
                    nc.vector.tensor_tensor(rawn[:], sc_ps[:], recip[:],
                                            mybir.AluOpType.mult)
                    nc.scalar.add(out_sb[:, i0:i0 + IH], rawn[:], b2_sb[:])

            nc.sync.dma_start(out[:], out_sb[:])
            if exchange:
                # all sends flushed before teardown: 15 preps x 16
                w = nc.gpsimd.wait_ge(lsem, 0)
                patches.append((w, 15 * 16))

    if exchange:
        nc.has_collectives = True   # force NRT global comm init for RDMA
    for w, v in patches:
        w.ins.sync_info.on_wait[0].wait_value = v
    if my_sems:
        nc.clear_and_free_semaphores(my_sems)
    nc.finalize()
    return nc


def _prep(inputs):
    """Host-side layout prep: transposes, fp8 casts, Wv@W1 fold."""
    f32 = np.float32
    bf16 = ml_dtypes.bfloat16
    fp8 = ml_dtypes.float8_e4m3
    x = np.ascontiguousarray(inputs["x"], dtype=f32)
    xt_r = np.ascontiguousarray(
        x.T.reshape(KC, P, N).transpose(1, 0, 2).astype(fp8))    # [P, KC, N]

    def w_r(w, scale, cols):  # [IN, cols] -> [P, KC, cols], fp8 pre-scaled
        return np.ascontiguousarray(
            (np.asarray(w, f32) * scale).reshape(KC, P, cols)
            .transpose(1, 0, 2).astype(fp8))

    wv_w1 = np.asarray(inputs["Wv"], f32) @ np.asarray(inputs["W1"], f32)
    b1pp = (np.asarray(inputs["bv"], f32) @ np.asarray(inputs["W1"], f32)
            + np.asarray(inputs["b1"], f32))                     # [256]

    shared = {
        "wq": w_r(inputs["Wq"], WS, D),
        "wk": w_r(inputs["Wk"], WS, D),
        "wv1": w_r(wv_w1, WSV, DP2),
        "bq": np.ascontiguousarray(
            (np.asarray(inputs["bq"], f32) * WS).reshape(DC, P).T),
        "bvp": np.ascontiguousarray(
            np.broadcast_to(b1pp * WSV, (P, 4, DP2)).astype(f32)),
        "w2": np.ascontiguousarray(
            (np.asarray(inputs["W2"], f32) * 0.5)
            .reshape(VC, P).T.astype(bf16)),
        "b2": np.asarray(inputs["b2"], f32).reshape(1, 1),
    }
    if not EXCHANGE:
        # chunk-major [NT, P, KC, JT] so each 512-col tile DMAs contiguously
        NT = N // JT
        shared["xt"] = np.ascontiguousarray(
            xt_r.reshape(P, KC, NT, JT).transpose(2, 0, 1, 3))
    xqs = [np.ascontiguousarray(xt_r[:, :, c * NB:(c + 1) * NB])
           for c in range(NCORES)]
    return shared, xqs


def kernel(**inputs) -> np.ndarray:
    if "nc" not in _CACHE:
        _CACHE["nc"] = _build()
    nc = _CACHE["nc"]
    shared, xqs = _prep(inputs)
    in_maps = [dict(shared, xq=xqs[c]) for c in range(NCORES)]
    res = run_bass_kernel_spmd(nc, in_maps, core_ids=list(range(NCORES)))
    return np.concatenate([res.results[c]["out"][0] for c in range(NCORES)])


# revision 26
# speedup vs baseline: 1.0122x; 1.0122x over previous
"""CAAN attention-scorer kernel for 8 Trainium2 NeuronCores (v2).

scores = relu(softmax(QK^T/sqrt(D)) @ V @ W1 + b1) @ W2 + b2
with Q/K/V = x @ W{q,k,v} + b{q,k,v};  N=8192, IN_DIM=1024, D=512.

Math restructure vs v1 (all exact up to fp8 rounding):
  * Wv@W1 folded on host (attention rows sum to 1):
      V'' = x @ (Wv W1) + (bv W1 + b1)  [N, 256]
      h   = relu(attn @ V'')            -- the whole first MLP layer is gone
  * bk dropped: a per-row constant shift of the logits cancels in softmax.
  * softmax normalization deferred past relu (relu(a)/c == relu(a/c), c>0)
    to the final [1, N] score vector: scores = (W2/2)^T relu(ctxU'')*recip.

Sharding: row-parallel attention; each core owns 1024 query rows. The
kT / V'' projections are REPLICATED on every core (EXCHANGE=False, the
shipped configuration): measured on this axon/PJRT runtime, the NRT
AllGather has a ~190us serial latency even for tiny payloads (it overlaps
compute but is useless as a data dependency), and SBUF-to-SBUF
remote_dma_broadcast does not deliver at all (sem-only broadcast timed
out and left the device NRT_EXEC_UNIT_UNRECOVERABLE) — so replicating
the 1/8-sized folded projections is the fastest working option. The
EXCHANGE=True path (XOR-addressed remote_dma_broadcast all-gather with a
ready-handshake) is kept for a runtime where SWDGE remote DMA works.

Tile's scheduling simulator cannot model remotely-incremented semaphores,
so waits on them are emitted with threshold 0 and patched to the real
value after TileContext exits (before finalize).

Layouts (x8 / x16 are exact power-of-two fp8 pre-scales):
  kT[d, j]   = (8 Wk)^T x^T             d on partitions   (fp8)
  qT[d, i]   = (8 Wq)^T xq^T (+8 bq)                      (fp8)
  v''[j, d'] = x (16 WvW1) (+16 b1'')   j on partitions   (fp8)
  S^T[j, i]  = kT-chunk^T qT            (= 64 * true)
  E          = exp(S^T / (64 sqrt(D)))  PSUM -> SBUF fp8, [P, 1024] ops
  ctxU''    += v''-chunk^T E            (= 16 * true)
  den[1,i]  += (8.0)^T E                (= 8 * sum E)
  raw[1, i]  = (W2/2)^T relu(ctxU'')    (= 8 * true raw)
  out        = raw * (1/den) + b2
"""

import numpy as np
import ml_dtypes

import concourse.tile as tile
from concourse import bacc, mybir
from concourse.bass_utils import run_bass_kernel_spmd

N, IN_DIM, D = 8192, 1024, 512
DP2 = D // 2                # 256 folded v'' width
NCORES = 8
NB = N // NCORES            # 1024 rows per core
P = 128
KC = IN_DIM // P            # 8 k-chunks of the input dim
KP = KC // 2                # 4 DoubleRow k-pairs
DC = D // P                 # 4 d-chunks (q/k)
DPAIR = DC // 2             # 2 DoubleRow d-pairs
VC = DP2 // P               # 2 d'-chunks (v'')
JT = 512                    # j-tile width in phase A
BJC = NB // P               # 8 j-chunks per block
BJP = BJC // 2              # 4 j-pairs per block
IH = 512                    # i-half width in phase B
NIH = NB // IH              # 2
WS = 8.0                    # fp8 weight pre-scale for Wq/Wk
WSV = 16.0                  # fp8 pre-scale for folded WvW1
SCALE = 1.0 / float(np.sqrt(np.float32(D))) / (WS * WS)

FP8 = mybir.dt.float8e4
BF16 = mybir.dt.bfloat16
F32 = mybir.dt.float32
DR = mybir.MatmulPerfMode.DoubleRow
ACT = mybir.ActivationFunctionType

EXCHANGE = False

_CACHE = {}


def _build(exchange=EXCHANGE):
    nc = bacc.Bacc(None, target_bir_lowering=False, debug=False,
                   num_devices=NCORES, use_seq_codegen=True)

    NT = N // JT
    xq = nc.declare_dram_parameter("xq", [P, KC, NB], FP8, isOutput=False)
    if not exchange:
        # chunk-major so each 512-column tile is one contiguous DMA
        xt = nc.declare_dram_parameter("xt", [NT, P, KC, JT], FP8,
                                       isOutput=False)
    wq = nc.declare_dram_parameter("wq", [P, KC, D], FP8, isOutput=False)
    wk = nc.declare_dram_parameter("wk", [P, KC, D], FP8, isOutput=False)
    wv1 = nc.declare_dram_parameter("wv1", [P, KC, DP2], FP8, isOutput=False)
    bq = nc.declare_dram_parameter("bq", [P, DC], F32, isOutput=False)   # x8
    bvp = nc.declare_dram_parameter("bvp", [P, 4, DP2], F32,
                                    isOutput=False)  # x16
    w2 = nc.declare_dram_parameter("w2", [P, VC], BF16, isOutput=False)  # x0.5
    b2 = nc.declare_dram_parameter("b2", [1, 1], F32, isOutput=False)
    out = nc.declare_dram_parameter("out", [1, NB], F32, isOutput=True)

    patches = []   # (BassInstruction, real_wait_value)
    my_sems = []
    with tile.TileContext(nc) as tc:
        with tc.tile_pool(name="singles", bufs=1) as singles:
            # ---- weights / constants into SBUF ----
            wk_sb = singles.tile([P, KC, D], FP8)
            wq_sb = singles.tile([P, KC, D], FP8)
            wv1_sb = singles.tile([P, KC, DP2], FP8)
            bq_sb = singles.tile([P, DC], F32)
            bvp_sb = singles.tile([P, 4, DP2], F32)
            w2_sb = singles.tile([P, VC], BF16)
            b2_sb = singles.tile([1, 1], F32)
            cs_w = singles.tile([P, 2, 32], FP8)   # colsum weights = 8.0
            xq_sb = singles.tile([P, KC, NB], FP8)
            if not exchange:
                xt_sb = singles.tile([P, NT, KC, JT], FP8)
            qt_sb = singles.tile([P, DC, NB], FP8)
            # gathered kT / v'': slot s = block of core (own XOR s)
            # (global block s when exchange=False)
            ktg = singles.tile([P, NCORES, DC, NB], FP8)
            vg = singles.tile([P, NCORES, BJC, DP2], FP8)
            out_sb = singles.tile([1, NB], F32)

            # DMA order tuned so Q can start immediately and kt follows:
            # xq (sync) + wq/bq (gpsimd) land first, wk next, then xt.
            for h in range(2):
                nc.sync.dma_start(xq_sb[:, 4 * h:4 * h + 4],
                                  xq[:, 4 * h:4 * h + 4])
            for h in range(2):
                nc.gpsimd.dma_start(out=wq_sb[:, 4 * h:4 * h + 4],
                                    in_=wq[:, 4 * h:4 * h + 4])
            nc.gpsimd.dma_start(out=bq_sb[:], in_=bq[:])
            for h in range(2):
                nc.gpsimd.dma_start(out=wk_sb[:, 4 * h:4 * h + 4],
                                    in_=wk[:, 4 * h:4 * h + 4])
            nc.gpsimd.dma_start(out=wv1_sb[:], in_=wv1[:])
            nc.gpsimd.dma_start(out=bvp_sb[:], in_=bvp[:])
            nc.gpsimd.dma_start(out=w2_sb[:], in_=w2[:])
            nc.gpsimd.dma_start(out=b2_sb[:], in_=b2[:])
            nc.vector.memset(cs_w[:], WS)
            if not exchange:
                for t in range(NT):
                    nc.sync.dma_start(xt_sb[:, t], xt[t])

            if exchange:
                rsems = [nc.alloc_semaphore(f"xch_{m}")
                         for m in range(1, NCORES)]
                ready = nc.alloc_semaphore("xch_ready")
                lsem = nc.alloc_semaphore("xch_local")
                my_sems = rsems + [ready, lsem]
                margin_dram = nc.dram_tensor("margin_scratch", [1, 2], FP8)

            # j-source slices for the kT / v'' projections
            if exchange:
                kv_blocks = 1

                def k_mov(t, kp):
                    return xq_sb[:, 2 * kp:2 * kp + 2,
                                 (t % 2) * JT:(t % 2 + 1) * JT]

                def v_mov(t, kp, q):
                    jc = (t % 2) * 4 + q
                    return xq_sb[:, 2 * kp:2 * kp + 2, jc * P:(jc + 1) * P]
            else:
                kv_blocks = NCORES

                def k_mov(t, kp):
                    return xt_sb[:, t, 2 * kp:2 * kp + 2]

                def v_mov(t, kp, q):
                    return xt_sb[:, t, 2 * kp:2 * kp + 2, q * P:(q + 1) * P]

            with (
                tc.tile_pool(name="st2", bufs=2, space="PSUM") as st2_pool,
                tc.tile_pool(name="ctxp", bufs=1, space="PSUM") as ctx_pool,
                tc.tile_pool(name="csp", bufs=1, space="PSUM") as cs_pool,
                tc.tile_pool(name="scp", bufs=1, space="PSUM") as sc_pool,
                tc.tile_pool(name="etile", bufs=6) as etile,
                tc.tile_pool(name="mlp", bufs=2) as mlp,
            ):
                # ---- phase A ----
                def emit_q():
                    # qT (+8 bq): only needs xq/wq
                    for it in range(NB // JT):
                        for dcp in range(DPAIR):
                            ps = st2_pool.tile([P, 2, JT], F32, tag="st")
                            for half in range(2):
                                dc = 2 * dcp + half
                                for kp in range(KP):
                                    nc.tensor.matmul(
                                        ps[:, half],
                                        wq_sb[:, 2 * kp:2 * kp + 2,
                                              dc * P:(dc + 1) * P],
                                        xq_sb[:, 2 * kp:2 * kp + 2,
                                              it * JT:(it + 1) * JT],
                                        start=(kp == 0), stop=(kp == KP - 1),
                                        perf_mode=DR)
                            for half in range(2):
                                dc = 2 * dcp + half
                                nc.scalar.activation(
                                    qt_sb[:, dc, it * JT:(it + 1) * JT],
                                    ps[:, half], ACT.Identity,
                                    bias=bq_sb[:, dc:dc + 1], scale=1.0)

                def emit_kv():
                    # kT / v'' per 512-column tile, in xt DMA order
                    for b in range(kv_blocks):
                        for jh in range(NB // JT):
                            t = 2 * b + jh
                            # kT (no bias; bk cancels in softmax)
                            for dcp in range(DPAIR):
                                ps = st2_pool.tile([P, 2, JT], F32, tag="st")
                                for half in range(2):
                                    dc = 2 * dcp + half
                                    for kp in range(KP):
                                        nc.tensor.matmul(
                                            ps[:, half],
                                            wk_sb[:, 2 * kp:2 * kp + 2,
                                                  dc * P:(dc + 1) * P],
                                            k_mov(t, kp),
                                            start=(kp == 0),
                                            stop=(kp == KP - 1),
                                            perf_mode=DR)
                                nc.scalar.activation(
                                    ktg[:, b, 2 * dcp:2 * dcp + 2,
                                        jh * JT:(jh + 1) * JT],
                                    ps[:], ACT.Copy, bias=0.0, scale=1.0)
                            # v'' for the same 4 j-chunks
                            ps = st2_pool.tile([P, 4, DP2], F32, tag="st")
                            for q in range(4):
                                for kp in range(KP):
                                    nc.tensor.matmul(
                                        ps[:, q],
                                        v_mov(t, kp, q),
                                        wv1_sb[:, 2 * kp:2 * kp + 2],
                                        start=(kp == 0), stop=(kp == KP - 1),
                                        perf_mode=DR)
                            nc.vector.tensor_tensor(
                                vg[:, b, 4 * jh:4 * jh + 4], ps[:],
                                bvp_sb[:], mybir.AluOpType.add)

                if exchange:
                    # kv shards first so the sends fire early; Q fills the
                    # window while data is in flight
                    emit_kv()
                    # ready handshake (NO self-send) + 14 XOR sends
                    nc.gpsimd.dma_start(margin_dram[:], ktg[0:1, 0, 0, 0:2])
                    nc.gpsimd.remote_sem_update_broadcast(
                        remote_sem=ready, local_sem=lsem,
                        rdests=[None] + [(0, k) for k in range(1, NCORES)])
                    nc.gpsimd.trigger_dma(count=1)
                    for m in range(1, NCORES):
                        rdests = [None] * 8
                        rdests[m] = (0, m)
                        nc.gpsimd.remote_dma_broadcast(
                            ktg[:, m], ktg[:, 0],
                            remote_sem=rsems[m - 1], local_sem=lsem,
                            rdests=rdests)
                        nc.gpsimd.remote_dma_broadcast(
                            vg[:, m], vg[:, 0],
                            remote_sem=rsems[m - 1], local_sem=lsem,
                            rdests=rdests)
                    # 7 peers x (+2) = 14: everyone (but me) has entered
                    w = nc.gpsimd.wait_ge(ready, 0)
                    patches.append((w, 2 * (NCORES - 1)))
                    nc.gpsimd.trigger_dma(count=None)
                    emit_q()
                else:
                    emit_q()
                    emit_kv()

                # ---- phase B: attention over the 8 blocks ----
                for ih in range(NIH):
                    i0 = ih * IH
                    ctx_ps = ctx_pool.tile([P, VC, IH], F32)
                    cs_ps = cs_pool.tile([32, IH], F32)
                    for s in range(NCORES):
                        if exchange and ih == 0 and s >= 1:
                            w = nc.tensor.wait_ge(rsems[s - 1], 0)
                            patches.append((w, 4))
                        for tp in range(BJP):
                            st = st2_pool.tile([P, 2, IH], F32, tag="st")
                            for half in range(2):
                                jc = 2 * tp + half
                                for dp in range(DPAIR):
                                    nc.tensor.matmul(
                                        st[:, half],
                                        ktg[:, s, 2 * dp:2 * dp + 2,
                                            jc * P:(jc + 1) * P],
                                        qt_sb[:, 2 * dp:2 * dp + 2,
                                              i0:i0 + IH],
                                        start=(dp == 0),
                                        stop=(dp == DPAIR - 1),
                                        perf_mode=DR)
                            e_t = etile.tile([P, 2, IH], FP8, tag="et")
                            nc.scalar.activation(e_t[:], st[:], ACT.Exp,
                                                 bias=0.0, scale=SCALE)
                            first = (s == 0 and tp == 0)
                            last = (s == NCORES - 1 and tp == BJP - 1)
                            nc.tensor.matmul(cs_ps[:], cs_w[:], e_t[:],
                                             start=first, stop=last,
                                             perf_mode=DR)
                            for vc in range(VC):
                                nc.tensor.matmul(
                                    ctx_ps[:, vc],
                                    vg[:, s, 2 * tp:2 * tp + 2,
                                       vc * P:(vc + 1) * P],
                                    e_t[:],
                                    start=first, stop=last,
                                    perf_mode=DR)

                    # tail: h = relu(ctxU''); raw = (W2/2)^T h; out = raw/den
                    h_sb = mlp.tile([P, VC, IH], BF16, tag="hsb")
                    sc_ps = sc_pool.tile([1, IH], F32, tag="sc")
                    for vc in range(VC):
                        nc.scalar.activation(h_sb[:, vc], ctx_ps[:, vc],
                                             ACT.Relu, bias=0.0, scale=1.0)
                        nc.tensor.matmul(sc_ps[:], w2_sb[:, vc:vc + 1],
                                         h_sb[:, vc],
                                         start=(vc == 0), stop=(vc == VC - 1))
                    recip = mlp.tile([1, IH], F32, tag="recip")
                    nc.vector.reciprocal_approx_fast(recip[:], cs_ps[0:1])
                    rawn = mlp.tile([1, IH], F32, tag="rawn")
                    nc.vector.tensor_tensor(rawn[:], sc_ps[:], recip[:],
                                            mybir.AluOpType.mult)
                    nc.scalar.add(out_sb[:, i0:i0 + IH], rawn[:], b2_sb[:])

            nc.sync.dma_start(out[:], out_sb[:])
            if exchange:
                # all sends flushed before teardown: 15 preps x 16
                w = nc.gpsimd.wait_ge(lsem, 0)
                patches.append((w, 15 * 16))

    if exchange:
        nc.has_collectives = True   # force NRT global comm init for RDMA
    for w, v in patches:
        w.ins.sync_info.on_wait[0].wait_value = v
    if my_sems:
        nc.clear_and_free_semaphores(my_sems)
    nc.finalize()
    return nc


def _prep(inputs):
    """Host-side layout prep: transposes, fp8 casts, Wv@W1 fold."""
    f32 = np.float32
    bf16 = ml_dtypes.bfloat16
    fp8 = ml_dtypes.float8_e4m3
    x = np.ascontiguousarray(inputs["x"], dtype=f32)
    xt_r = np.ascontiguousarray(
        x.T.reshape(KC, P, N).transpose(1, 0, 2).astype(fp8))    # [P, KC, N]

    def w_r(w, scale, cols):  # [IN, cols] -> [P, KC, cols], fp8 pre-scaled
        return np.ascontiguousarray(
            (np.asarray(w, f32) * scale).reshape(KC, P, cols)
            .transpose(1, 0, 2).astype(fp8))

    wv_w1 = np.asarray(inputs["Wv"], f32) @ np.asarray(inputs["W1"], f32)
    b1pp = (np.asarray(inputs["bv"], f32) @ np.asarray(inputs["W1"], f32)
            + np.asarray(inputs["b1"], f32))                     # [256]

    shared = {
        "wq": w_r(inputs["Wq"], WS, D),
        "wk": w_r(inputs["Wk"], WS, D),
        "wv1": w_r(wv_w1, WSV, DP2),
        "bq": np.ascontiguousarray(
            (np.asarray(inputs["bq"], f32) * WS).reshape(DC, P).T),
        "bvp": np.ascontiguousarray(
            np.broadcast_to(b1pp * WSV, (P, 4, DP2)).astype(f32)),
        "w2": np.ascontiguousarray(
            (np.asarray(inputs["W2"], f32) * 0.5)
            .reshape(VC, P).T.astype(bf16)),
        "b2": np.asarray(inputs["b2"], f32).reshape(1, 1),
    }
    if not EXCHANGE:
        # chunk-major [NT, P, KC, JT] so each 512-col tile DMAs contiguously
        NT = N // JT
        shared["xt"] = np.ascontiguousarray(
            xt_r.reshape(P, KC, NT, JT).transpose(2, 0, 1, 3))
    xqs = [np.ascontiguousarray(xt_r[:, :, c * NB:(c + 1) * NB])
           for c in range(NCORES)]
    return shared, xqs


def kernel(**inputs) -> np.ndarray:
    if "nc" not in _CACHE:
        _CACHE["nc"] = _build()
    nc = _CACHE["nc"]
    shared, xqs = _prep(inputs)
    in_maps = [dict(shared, xq=xqs[c]) for c in range(NCORES)]
    res = run_bass_kernel_spmd(nc, in_maps, core_ids=list(range(NCORES)))
    return np.concatenate([res.results[c]["out"][0] for c in range(NCORES)])


# revision 28
# speedup vs baseline: 1.0144x; 1.0022x over previous
"""CAAN attention-scorer kernel for 8 Trainium2 NeuronCores (v2).

scores = relu(softmax(QK^T/sqrt(D)) @ V @ W1 + b1) @ W2 + b2
with Q/K/V = x @ W{q,k,v} + b{q,k,v};  N=8192, IN_DIM=1024, D=512.

Math restructure vs v1 (all exact up to fp8 rounding):
  * Wv@W1 folded on host (attention rows sum to 1):
      V'' = x @ (Wv W1) + (bv W1 + b1)  [N, 256]
      h   = relu(attn @ V'')            -- the whole first MLP layer is gone
  * bk dropped: a per-row constant shift of the logits cancels in softmax.
  * softmax normalization deferred past relu (relu(a)/c == relu(a/c), c>0)
    to the final [1, N] score vector: scores = (W2/2)^T relu(ctxU'')*recip.

Sharding: row-parallel attention; each core owns 1024 query rows. The
kT / V'' projections are REPLICATED on every core (EXCHANGE=False, the
shipped configuration): measured on this axon/PJRT runtime, the NRT
AllGather has a ~190us serial latency even for tiny payloads (it overlaps
compute but is useless as a data dependency), and SBUF-to-SBUF
remote_dma_broadcast does not deliver at all (sem-only broadcast timed
out and left the device NRT_EXEC_UNIT_UNRECOVERABLE) — so replicating
the 1/8-sized folded projections is the fastest working option. The
EXCHANGE=True path (XOR-addressed remote_dma_broadcast all-gather with a
ready-handshake) is kept for a runtime where SWDGE remote DMA works.

Tile's scheduling simulator cannot model remotely-incremented semaphores,
so waits on them are emitted with threshold 0 and patched to the real
value after TileContext exits (before finalize).

Layouts (x8 / x16 are exact power-of-two fp8 pre-scales):
  kT[d, j]   = (8 Wk)^T x^T             d on partitions   (fp8)
  qT[d, i]   = (8 Wq)^T xq^T (+8 bq)                      (fp8)
  v''[j, d'] = x (16 WvW1) (+16 b1'')   j on partitions   (fp8)
  S^T[j, i]  = kT-chunk^T qT            (= 64 * true)
  E          = exp(S^T / (64 sqrt(D)))  PSUM -> SBUF fp8, [P, 1024] ops
  ctxU''    += v''-chunk^T E            (= 16 * true)
  den[1,i]  += (8.0)^T E                (= 8 * sum E)
  raw[1, i]  = (W2/2)^T relu(ctxU'')    (= 8 * true raw)
  out        = raw * (1/den) + b2
"""

import numpy as np
import ml_dtypes

import concourse.tile as tile
from concourse import bacc, mybir
from concourse.bass_utils import run_bass_kernel_spmd

N, IN_DIM, D = 8192, 1024, 512
DP2 = D // 2                # 256 folded v'' width
NCORES = 8
NB = N // NCORES            # 1024 rows per core
P = 128
KC = IN_DIM // P            # 8 k-chunks of the input dim
KP = KC // 2                # 4 DoubleRow k-pairs
DC = D // P                 # 4 d-chunks (q/k)
DPAIR = DC // 2             # 2 DoubleRow d-pairs
VC = DP2 // P               # 2 d'-chunks (v'')
JT = 512                    # j-tile width in phase A
BJC = NB // P               # 8 j-chunks per block
BJP = BJC // 2              # 4 j-pairs per block
IH = 512                    # i-half width in phase B
NIH = NB // IH              # 2
WS = 8.0                    # fp8 weight pre-scale for Wq/Wk
WSV = 16.0                  # fp8 pre-scale for folded WvW1
SCALE = 1.0 / float(np.sqrt(np.float32(D))) / (WS * WS)

FP8 = mybir.dt.float8e4
BF16 = mybir.dt.bfloat16
F32 = mybir.dt.float32
DR = mybir.MatmulPerfMode.DoubleRow
ACT = mybir.ActivationFunctionType

EXCHANGE = False

_CACHE = {}


def _build(exchange=EXCHANGE):
    nc = bacc.Bacc(None, target_bir_lowering=False, debug=False,
                   num_devices=NCORES, use_seq_codegen=True)

    NT = N // JT
    xq = nc.declare_dram_parameter("xq", [P, KC, NB], FP8, isOutput=False)
    if not exchange:
        # chunk-major so each 512-column tile is one contiguous DMA
        xt = nc.declare_dram_parameter("xt", [NT, P, KC, JT], FP8,
                                       isOutput=False)
    wq = nc.declare_dram_parameter("wq", [P, KC, D], FP8, isOutput=False)
    wk = nc.declare_dram_parameter("wk", [P, KC, D], FP8, isOutput=False)
    wv1 = nc.declare_dram_parameter("wv1", [P, KC, DP2], FP8, isOutput=False)
    bq = nc.declare_dram_parameter("bq", [P, DC], F32, isOutput=False)   # x8
    bvp = nc.declare_dram_parameter("bvp", [P, 4, DP2], F32,
                                    isOutput=False)  # x16
    w2 = nc.declare_dram_parameter("w2", [P, VC], BF16, isOutput=False)  # x0.5
    b2 = nc.declare_dram_parameter("b2", [1, 1], F32, isOutput=False)
    out = nc.declare_dram_parameter("out", [1, NB], F32, isOutput=True)

    patches = []   # (BassInstruction, real_wait_value)
    my_sems = []
    with tile.TileContext(nc) as tc:
        with tc.tile_pool(name="singles", bufs=1) as singles:
            # ---- weights / constants into SBUF ----
            wk_sb = singles.tile([P, KC, D], FP8)
            wq_sb = singles.tile([P, KC, D], FP8)
            wv1_sb = singles.tile([P, KC, DP2], FP8)
            bq_sb = singles.tile([P, DC], F32)
            bvp_sb = singles.tile([P, 4, DP2], F32)
            w2_sb = singles.tile([P, VC], BF16)
            b2_sb = singles.tile([1, 1], F32)
            cs_w = singles.tile([P, 2, 32], FP8)   # colsum weights = 8.0
            xq_sb = singles.tile([P, KC, NB], FP8)
            if not exchange:
                xt_sb = singles.tile([P, NT, KC, JT], FP8)
            qt_sb = singles.tile([P, DC, NB], FP8)
            # gathered kT / v'': slot s = block of core (own XOR s)
            # (global block s when exchange=False)
            ktg = singles.tile([P, NCORES, DC, NB], FP8)
            vg = singles.tile([P, NCORES, BJC, DP2], FP8)
            out_sb = singles.tile([1, NB], F32)

            # DMA order tuned so Q can start immediately and kt follows:
            # xq (sync) + wq/bq (gpsimd) land first, wk next, then xt.
            for h in range(2):
                nc.sync.dma_start(xq_sb[:, 4 * h:4 * h + 4],
                                  xq[:, 4 * h:4 * h + 4])
            for h in range(2):
                nc.gpsimd.dma_start(out=wq_sb[:, 4 * h:4 * h + 4],
                                    in_=wq[:, 4 * h:4 * h + 4])
            nc.gpsimd.dma_start(out=bq_sb[:], in_=bq[:])
            for h in range(2):
                nc.gpsimd.dma_start(out=wk_sb[:, 4 * h:4 * h + 4],
                                    in_=wk[:, 4 * h:4 * h + 4])
            nc.gpsimd.dma_start(out=wv1_sb[:], in_=wv1[:])
            nc.gpsimd.dma_start(out=bvp_sb[:], in_=bvp[:])
            nc.gpsimd.dma_start(out=w2_sb[:], in_=w2[:])
            nc.gpsimd.dma_start(out=b2_sb[:], in_=b2[:])
            nc.vector.memset(cs_w[:], WS)
            if not exchange:
                for t in range(NT):
                    nc.sync.dma_start(xt_sb[:, t], xt[t])

            if exchange:
                rsems = [nc.alloc_semaphore(f"xch_{m}")
                         for m in range(1, NCORES)]
                ready = nc.alloc_semaphore("xch_ready")
                lsem = nc.alloc_semaphore("xch_local")
                my_sems = rsems + [ready, lsem]
                margin_dram = nc.dram_tensor("margin_scratch", [1, 2], FP8)

            # j-source slices for the kT / v'' projections
            if exchange:
                kv_blocks = 1

                def k_mov(t, kp):
                    return xq_sb[:, 2 * kp:2 * kp + 2,
                                 (t % 2) * JT:(t % 2 + 1) * JT]

                def v_mov(t, kp, q):
                    jc = (t % 2) * 4 + q
                    return xq_sb[:, 2 * kp:2 * kp + 2, jc * P:(jc + 1) * P]
            else:
                kv_blocks = NCORES

                def k_mov(t, kp):
                    return xt_sb[:, t, 2 * kp:2 * kp + 2]

                def v_mov(t, kp, q):
                    return xt_sb[:, t, 2 * kp:2 * kp + 2, q * P:(q + 1) * P]

            with (
                tc.tile_pool(name="st2", bufs=2, space="PSUM") as st2_pool,
                tc.tile_pool(name="ctxp", bufs=1, space="PSUM") as ctx_pool,
                tc.tile_pool(name="csp", bufs=1, space="PSUM") as cs_pool,
                tc.tile_pool(name="scp", bufs=1, space="PSUM") as sc_pool,
                tc.tile_pool(name="etile", bufs=6) as etile,
                tc.tile_pool(name="mlp", bufs=2) as mlp,
            ):
                # ---- PE clock warm-up ----
                # The PE p-state ramps to 2.4GHz only after ~3us of
                # continuous execution; real matmuls can't start until the
                # first weights/input land (~12us). Burn dummy matmuls on a
                # memset tile during that window so phase A runs warm.
                warm = singles.tile([P, 2, JT], FP8)
                nc.vector.memset(warm[:], 0.25)
                for _ in range(12):
                    ps = st2_pool.tile([P, 2, JT], F32, tag="st")
                    nc.tensor.matmul(ps[:, 0], warm[:, :, 0:P], warm[:],
                                     start=True, stop=True, perf_mode=DR)

                # ---- phase A ----
                def emit_q():
                    # qT (+8 bq): only needs xq/wq
                    for it in range(NB // JT):
                        for dcp in range(DPAIR):
                            ps = st2_pool.tile([P, 2, JT], F32, tag="st")
                            for half in range(2):
                                dc = 2 * dcp + half
                                for kp in range(KP):
                                    nc.tensor.matmul(
                                        ps[:, half],
                                        wq_sb[:, 2 * kp:2 * kp + 2,
                                              dc * P:(dc + 1) * P],
                                        xq_sb[:, 2 * kp:2 * kp + 2,
                                              it * JT:(it + 1) * JT],
                                        start=(kp == 0), stop=(kp == KP - 1),
                                        perf_mode=DR)
                            for half in range(2):
                                dc = 2 * dcp + half
                                nc.scalar.activation(
                                    qt_sb[:, dc, it * JT:(it + 1) * JT],
                                    ps[:, half], ACT.Identity,
                                    bias=bq_sb[:, dc:dc + 1], scale=1.0)

                def emit_kv():
                    # kT / v'' per 512-column tile, in xt DMA order
                    for b in range(kv_blocks):
                        for jh in range(NB // JT):
                            t = 2 * b + jh
                            # kT (no bias; bk cancels in softmax)
                            for dcp in range(DPAIR):
                                ps = st2_pool.tile([P, 2, JT], F32, tag="st")
                                for half in range(2):
                                    dc = 2 * dcp + half
                                    for kp in range(KP):
                                        nc.tensor.matmul(
                                            ps[:, half],
                                            wk_sb[:, 2 * kp:2 * kp + 2,
                                                  dc * P:(dc + 1) * P],
                                            k_mov(t, kp),
                                            start=(kp == 0),
                                            stop=(kp == KP - 1),
                                            perf_mode=DR)
                                nc.scalar.activation(
                                    ktg[:, b, 2 * dcp:2 * dcp + 2,
                                        jh * JT:(jh + 1) * JT],
                                    ps[:], ACT.Copy, bias=0.0, scale=1.0)
                            # v'' for the same 4 j-chunks
                            ps = st2_pool.tile([P, 4, DP2], F32, tag="st")
                            for q in range(4):
                                for kp in range(KP):
                                    nc.tensor.matmul(
                                        ps[:, q],
                                        v_mov(t, kp, q),
                                        wv1_sb[:, 2 * kp:2 * kp + 2],
                                        start=(kp == 0), stop=(kp == KP - 1),
                                        perf_mode=DR)
                            nc.vector.tensor_tensor(
                                vg[:, b, 4 * jh:4 * jh + 4], ps[:],
                                bvp_sb[:], mybir.AluOpType.add)

                if exchange:
                    # kv shards first so the sends fire early; Q fills the
                    # window while data is in flight
                    emit_kv()
                    # ready handshake (NO self-send) + 14 XOR sends
                    nc.gpsimd.dma_start(margin_dram[:], ktg[0:1, 0, 0, 0:2])
                    nc.gpsimd.remote_sem_update_broadcast(
                        remote_sem=ready, local_sem=lsem,
                        rdests=[None] + [(0, k) for k in range(1, NCORES)])
                    nc.gpsimd.trigger_dma(count=1)
                    for m in range(1, NCORES):
                        rdests = [None] * 8
                        rdests[m] = (0, m)
                        nc.gpsimd.remote_dma_broadcast(
                            ktg[:, m], ktg[:, 0],
                            remote_sem=rsems[m - 1], local_sem=lsem,
                            rdests=rdests)
                        nc.gpsimd.remote_dma_broadcast(
                            vg[:, m], vg[:, 0],
                            remote_sem=rsems[m - 1], local_sem=lsem,
                            rdests=rdests)
                    # 7 peers x (+2) = 14: everyone (but me) has entered
                    w = nc.gpsimd.wait_ge(ready, 0)
                    patches.append((w, 2 * (NCORES - 1)))
                    nc.gpsimd.trigger_dma(count=None)
                    emit_q()
                else:
                    emit_q()
                    emit_kv()

                # ---- phase B: attention over the 8 blocks ----
                for ih in range(NIH):
                    i0 = ih * IH
                    ctx_ps = ctx_pool.tile([P, VC, IH], F32)
                    cs_ps = cs_pool.tile([32, IH], F32)
                    for s in range(NCORES):
                        if exchange and ih == 0 and s >= 1:
                            w = nc.tensor.wait_ge(rsems[s - 1], 0)
                            patches.append((w, 4))
                        for tp in range(BJP):
                            st = st2_pool.tile([P, 2, IH], F32, tag="st")
                            for half in range(2):
                                jc = 2 * tp + half
                                for dp in range(DPAIR):
                                    nc.tensor.matmul(
                                        st[:, half],
                                        ktg[:, s, 2 * dp:2 * dp + 2,
                                            jc * P:(jc + 1) * P],
                                        qt_sb[:, 2 * dp:2 * dp + 2,
                                              i0:i0 + IH],
                                        start=(dp == 0),
                                        stop=(dp == DPAIR - 1),
                                        perf_mode=DR)
                            e_t = etile.tile([P, 2, IH], FP8, tag="et")
                            nc.scalar.activation(e_t[:], st[:], ACT.Exp,
                                                 bias=0.0, scale=SCALE)
                            first = (s == 0 and tp == 0)
                            last = (s == NCORES - 1 and tp == BJP - 1)
                            nc.tensor.matmul(cs_ps[:], cs_w[:], e_t[:],
                                             start=first, stop=last,
                                             perf_mode=DR)
                            for vc in range(VC):
                                nc.tensor.matmul(
                                    ctx_ps[:, vc],
                                    vg[:, s, 2 * tp:2 * tp + 2,
                                       vc * P:(vc + 1) * P],
                                    e_t[:],
                                    start=first, stop=last,
                                    perf_mode=DR)

                    # tail: h = relu(ctxU''); raw = (W2/2)^T h; out = raw/den
                    h_sb = mlp.tile([P, VC, IH], BF16, tag="hsb")
                    sc_ps = sc_pool.tile([1, IH], F32, tag="sc")
                    for vc in range(VC):
                        nc.scalar.activation(h_sb[:, vc], ctx_ps[:, vc],
                                             ACT.Relu, bias=0.0, scale=1.0)
                        nc.tensor.matmul(sc_ps[:], w2_sb[:, vc:vc + 1],
                                         h_sb[:, vc],
                                         start=(vc == 0), stop=(vc == VC - 1))
                    recip = mlp.tile([1, IH], F32, tag="recip")
                    nc.vector.reciprocal_approx_fast(recip[:], cs_ps[0:1])
                    rawn = mlp.tile([1, IH], F32, tag="rawn")
                    nc.vector.tensor_tensor(rawn[:], sc_ps[:], recip[:],
                                            mybir.AluOpType.mult)
                    nc.scalar.add(out_sb[:, i0:i0 + IH], rawn[:], b2_sb[:])
                    nc.sync.dma_start(out[:, i0:i0 + IH],
                                      out_sb[:, i0:i0 + IH])

            if exchange:
                # all sends flushed before teardown: 15 preps x 16
                w = nc.gpsimd.wait_ge(lsem, 0)
                patches.append((w, 15 * 16))

    if exchange:
        nc.has_collectives = True   # force NRT global comm init for RDMA
    for w, v in patches:
        w.ins.sync_info.on_wait[0].wait_value = v
    if my_sems:
        nc.clear_and_free_semaphores(my_sems)
    nc.finalize()
    return nc


def _prep(inputs):
    """Host-side layout prep: transposes, fp8 casts, Wv@W1 fold."""
    f32 = np.float32
    bf16 = ml_dtypes.bfloat16
    fp8 = ml_dtypes.float8_e4m3
    x = np.ascontiguousarray(inputs["x"], dtype=f32)
    xt_r = np.ascontiguousarray(
        x.T.reshape(KC, P, N).transpose(1, 0, 2).astype(fp8))    # [P, KC, N]

    def w_r(w, scale, cols):  # [IN, cols] -> [P, KC, cols], fp8 pre-scaled
        return np.ascontiguousarray(
            (np.asarray(w, f32) * scale).reshape(KC, P, cols)
            .transpose(1, 0, 2).astype(fp8))

    wv_w1 = np.asarray(inputs["Wv"], f32) @ np.asarray(inputs["W1"], f32)
    b1pp = (np.asarray(inputs["bv"], f32) @ np.asarray(inputs["W1"], f32)
            + np.asarray(inputs["b1"], f32))                     # [256]

    shared = {
        "wq": w_r(inputs["Wq"], WS, D),
        "wk": w_r(inputs["Wk"], WS, D),
        "wv1": w_r(wv_w1, WSV, DP2),
        "bq": np.ascontiguousarray(
            (np.asarray(inputs["bq"], f32) * WS).reshape(DC, P).T),
        "bvp": np.ascontiguousarray(
            np.broadcast_to(b1pp * WSV, (P, 4, DP2)).astype(f32)),
        "w2": np.ascontiguousarray(
            (np.asarray(inputs["W2"], f32) * 0.5)
            .reshape(VC, P).T.astype(bf16)),
        "b2": np.asarray(inputs["b2"], f32).reshape(1, 1),
    }
    if not EXCHANGE:
        # chunk-major [NT, P, KC, JT] so each 512-col tile DMAs contiguously
        NT = N // JT
        shared["xt"] = np.ascontiguousarray(
            xt_r.reshape(P, KC, NT, JT).transpose(2, 0, 1, 3))
    xqs = [np.ascontiguousarray(xt_r[:, :, c * NB:(c + 1) * NB])
           for c in range(NCORES)]
    return shared, xqs


def kernel(**inputs) -> np.ndarray:
    if "nc" not in _CACHE:
        _CACHE["nc"] = _build()
    nc = _CACHE["nc"]
    shared, xqs = _prep(inputs)
    in_maps = [dict(shared, xq=xqs[c]) for c in range(NCORES)]
    res = run_bass_kernel_spmd(nc, in_maps, core_ids=list(range(NCORES)))
    return np.concatenate([res.results[c]["out"][0] for c in range(NCORES)])
